# revision 39
# baseline (speedup 1.0000x reference)
"""Fused GroupNorm + multi-head self-attention + output projection for
nn_Attention_55619826483814 on 8 TRN2 NeuronCores.

Reference computation (shapes hardcoded):
  x: (4, 256, 64, 64) f32
  GroupNorm(1 group) over (C,H,W) per sample -> per-channel affine (gamma, beta)
  qkv = w_qkv @ xn  (384 = 3*4heads*32dim rows)
  per head: sim = (q*scale)^T k ; attn = softmax(sim, axis=j) ; out = attn @ v
  y = w_out @ out + b_out     -> (4, 256, 64, 64)

Sharding: 8 cores = 4 batches x 2 spatial halves. Core c handles batch c//2
and query positions [2048*(c%2), 2048*(c%2)+2048). Attention is permutation
invariant over key/value positions, so each core receives its batch image
with the spatial axis rolled so that ITS query half sits at columns 0..2047
(host-side np.roll = pure data movement). Keys/values/groupnorm stats use all
4096 positions. Each core computes its full (256, 2048) output slab; the host
concatenates. No collectives, no partition-id.

On-device dataflow per core (all matmul inputs bf16, fp32 accumulation):
  stats (DVE bn_stats/bn_aggr per 512-piece overlapping the x DMA + PE
  cross-partition sum; a bf16 HAM-warmup burst is emitted right after the
  stats matmuls so the in-order PE FIFO lifts the clock gate to 2.4 GHz
  just before the qkv projections)
  -> fold norm into per-channel a,d; cast x to bf16 xn = a*x + d
  -> Q,K packed channel-major (head h at partitions 32h..32h+31; the softmax
     SCALE is folded into exp's free scale operand) and V^T
     position-major with a ones column at slot 32 of every 33-wide head
     block (the attv col-tile then emits the softmax denominator as U row
     32 of its 33-partition output) via PE.
  -> attention, (t=i-tile, jc=j-chunk) loop:
     sim: 4 heads CONCURRENTLY via PE row tiling (tile_position=(32h,0),
       K=32 each) into two 2-bank PSUM tiles (h01 | h23), ~3x faster than
       serial K=32 matmuls.
     exp: split between ScalarE (exact Exp, bf16 out) and VectorE
       (Schraudolph approx: round(A*sim+B) via fp32->int16 convert, bitcast
       to bf16; |rel err| <= 3.3%, softmax ratio cancels most of it) with a
       static balance schedule -- both engines run flat out; this is the
       kernel's critical path (~33.5M exps/core, ~1 elem/lane/cycle each).
     attv: 2 heads CONCURRENTLY via PE col tiling (tile_position=(0,64q),
       M=33 each), accumulating U[den|ch, i] over jc in 2 PSUM banks.
  -> tail per t: evacuate U banks, spread denominators across partitions via
     K=1 indicator outer-products, one reciprocal, one multiply (GpSimd),
     y = w_out^T stationary @ attn^T + b_out -> DMA out. The whole attention
     runs as ONE flat 128-iteration loop: the attv backlog and the staged
     per-t tail carry across t boundaries so the PE stream never drains
     (a drain >3.4us re-throttles the HAM clock gate to 1.2 GHz, and at
     half clock the PE becomes the critical path). Tiny N=64 dummy matmuls
     pad the PE stream near boundaries; they write into the next sim tile
     BEFORE its start=True matmuls clear the bank, so they are free.

Measured: 262-272 us on 8 cores at full clock (baseline 496 us), rel err
7.6e-3 (gate 2e-2). Under the chip's P0 power-state downclock (sustained
load, 2.4->2.0 GHz) the same NEFF measures ~315 us; it recovers after a few
minutes idle. Engine balance in steady state: ScalarE ~90% (exact exp),
VectorE ~80% (Schraudolph exp + evac/normalize), PE ~80% (row-tiled sim +
col-tiled attv) -- co-critical at a ~1.26 us/iteration cadence.
"""

import sys

sys.path.insert(0, "/opt/trn_rl_repo")

import numpy as np

import concourse.bass as bass
import concourse.mybir as mybir
import concourse.tile as tile
from concourse import bacc
from concourse.masks import make_identity

DT = mybir.dt
F32 = DT.float32
BF16 = DT.bfloat16
I16 = DT.int16
ALU = mybir.AluOpType
ACTF = mybir.ActivationFunctionType

DIM = 256  # channels
N = 4096  # spatial positions
NH = 2048  # per-core query half
HEADS = 4
DH = 32  # head dim
HID = 128
SCALE = DH ** -0.5
EPS = 1e-5
NTOT = DIM * N

N_CORES = 8

# Schraudolph exp in bf16 bit domain: bits = round(A*x + B); DVE fp32->int16
# conversion rounds to nearest (hardware-verified). C=5.5 centers the
# sawtooth error at +-3.3%.
SCHR_A = float(2.0 ** 7 / np.log(2.0))
SCHR_B = float(127.0 * 128 - 5.5)

# Fraction of exp tiles handled by ScalarE (exact); rest go to VectorE
# (Schraudolph). Balances the two engines' total busy time.
ACT_FRAC = 0.60


def _exp_schedule(n_tiles):
    sched = []
    acc = 0.0
    for _ in range(n_tiles):
        acc += ACT_FRAC
        if acc >= 1.0:
            sched.append(True)
            acc -= 1.0
        else:
            sched.append(False)
    return sched


DEBUG = False


def build_nc():
    nc = bacc.Bacc("TRN2", target_bir_lowering=False)

    xr_d = nc.dram_tensor("xr", [DIM, N], F32, kind="ExternalInput")
    wq_d = nc.dram_tensor("wq", [3 * HID, DIM], F32, kind="ExternalInput")
    wo_d = nc.dram_tensor("wo", [DIM, HID], F32, kind="ExternalInput")
    bo_d = nc.dram_tensor("bo", [DIM, 1], F32, kind="ExternalInput")
    gam_d = nc.dram_tensor("gam", [DIM, 1], F32, kind="ExternalInput")
    bet_d = nc.dram_tensor("bet", [DIM, 1], F32, kind="ExternalInput")
    y_d = nc.dram_tensor("y", [DIM, NH], F32, kind="ExternalOutput")
    if DEBUG:
        dbg_kp = nc.dram_tensor("dbg_kp", [128, N], BF16, kind="ExternalOutput")
        dbg_qp = nc.dram_tensor("dbg_qp", [128, NH], BF16, kind="ExternalOutput")
        dbg_vt = nc.dram_tensor("dbg_vt", [128, 32 * 132], BF16, kind="ExternalOutput")
        dbg_sim = nc.dram_tensor("dbg_sim", [128, 2048], F32, kind="ExternalOutput")
        dbg_ex = nc.dram_tensor("dbg_ex", [128, 2048], BF16, kind="ExternalOutput")
        dbg_uev = nc.dram_tensor("dbg_uev", [128, 1024], BF16, kind="ExternalOutput")
        dbg_rsb = nc.dram_tensor("dbg_rsb", [128, 512], F32, kind="ExternalOutput")
        dbg_at = nc.dram_tensor("dbg_at", [128, 512], BF16, kind="ExternalOutput")

    with tile.TileContext(nc) as tc:
        with (
            tc.tile_pool(name="small", bufs=1) as small,
            tc.tile_pool(name="big", bufs=1) as big,
            tc.tile_pool(name="pxf", bufs=2) as pxf,
            tc.tile_pool(name="pjunk", bufs=1) as pjunk,
            tc.tile_pool(name="pwst", bufs=3) as pwst,
            tc.tile_pool(name="pwost", bufs=2) as pwost,
            tc.tile_pool(name="ptiny", bufs=2) as ptiny,
            tc.tile_pool(name="puev", bufs=2) as puev,
            tc.tile_pool(name="pexp", bufs=12) as pexp,
            tc.tile_pool(name="pysb", bufs=4) as pysb,
            tc.tile_pool(name="simp", bufs=3, space="PSUM") as simp,
            tc.tile_pool(name="up", bufs=2, space="PSUM") as up,
        ):
            # ---------- constants ----------
            identity = small.tile([128, 128], F32, tag="ident")
            make_identity(nc, identity[:])
            ones128x32 = small.tile([128, 32], F32, tag="o12832")
            nc.gpsimd.memset(ones128x32[:], 1.0)
            ones1x128 = small.tile([1, 128], F32, tag="o1128")
            nc.gpsimd.memset(ones1x128[:], 1.0)
            # indicator rows e_h[0, m] = (m // 32 == h): a K=1 matmul with
            # lhsT=e_h replicates a (1, n) rhs onto partitions 32h..32h+31
            e_h = []
            for h in range(HEADS):
                e = small.tile([1, 128], BF16, tag=f"e{h}", name=f"e{h}")
                nc.gpsimd.memset(e[:], 0.0)
                nc.gpsimd.memset(e[:, 32 * h : 32 * h + 32], 1.0)
                e_h.append(e)

            gam_c, bet_c, bo_c = [], [], []
            for kc in range(2):
                g = small.tile([128, 1], F32, tag=f"gam{kc}", name=f"gam{kc}")
                nc.sync.dma_start(g[:], gam_d[128 * kc : 128 * kc + 128, :])
                gam_c.append(g)
                bt = small.tile([128, 1], F32, tag=f"bet{kc}", name=f"bet{kc}")
                nc.sync.dma_start(bt[:], bet_d[128 * kc : 128 * kc + 128, :])
                bet_c.append(bt)
                bb = small.tile([128, 1], F32, tag=f"bo{kc}", name=f"bo{kc}")
                nc.sync.dma_start(bb[:], bo_d[128 * kc : 128 * kc + 128, :])
                bo_c.append(bb)

            # ---------- load x (4 pieces per chunk so stats overlap DMA) ----------
            xf = []
            for kc in range(2):
                t = pxf.tile([128, N], F32, tag="xf", name=f"xf{kc}")
                for p in range(4):
                    nc.sync.dma_start(
                        t[:, 1024 * p : 1024 * p + 1024],
                        xr_d[128 * kc : 128 * kc + 128, 1024 * p : 1024 * p + 1024])
                xf.append(t)

            # ---------- weight transposes ----------
            wqkvT = [big.tile([128, 384], BF16, tag=f"wqkvT{c}", name=f"wqkvT{c}") for c in range(2)]
            for r in range(3):
                wst = pwst.tile([128, DIM], F32, tag="wst")
                nc.sync.dma_start(wst[:], wq_d[128 * r : 128 * r + 128, :])
                for c in range(2):
                    tp = up.tile([128, 128], F32, tag="u")
                    nc.tensor.transpose(tp[:], wst[:, 128 * c : 128 * c + 128], identity[:])
                    nc.vector.tensor_copy(wqkvT[c][:, 128 * r : 128 * r + 128], tp[:])
            woT = big.tile([128, DIM], BF16, tag="woT")
            for r in range(2):
                wst = pwost.tile([128, HID], F32, tag="wost")
                nc.sync.dma_start(wst[:], wo_d[128 * r : 128 * r + 128, :])
                tp = up.tile([128, 128], F32, tag="u")
                nc.tensor.transpose(tp[:], wst[:], identity[:])
                nc.vector.tensor_copy(woT[:, 128 * r : 128 * r + 128], tp[:])

            # ---------- groupnorm stats (bn_stats: one DVE pass/piece) ----------
            bno = ptiny.tile([128, 2, 8, 6], F32, tag="bno")
            for kc in range(2):
                for p in range(8):
                    piece = xf[kc][:, 512 * p : 512 * p + 512]
                    nc.vector.bn_stats(bno[:, kc, p], piece)
            # per-channel [mean, var] over each 4096-wide half
            stk = ptiny.tile([128, 4], F32, tag="stk")  # [m0, s0, m1, s1]
            for kc in range(2):
                mv = ptiny.tile([128, 2], F32, tag="mv", name=f"mv{kc}")
                nc.vector.bn_aggr(mv[:], bno[:, kc])
                nc.vector.tensor_copy(stk[:, 2 * kc : 2 * kc + 1], mv[:, 0:1])
                # s = E[x^2] = var + mean^2
                msq = ptiny.tile([128, 1], F32, tag="msq", name=f"msq{kc}")
                nc.vector.tensor_tensor(msq[:], mv[:, 0:1], mv[:, 0:1], ALU.mult)
                nc.vector.tensor_tensor(stk[:, 2 * kc + 1 : 2 * kc + 2],
                                        mv[:, 1:2], msq[:], ALU.add)
            stp = up.tile([32, 4], F32, tag="u")
            nc.tensor.matmul(stp[:], lhsT=ones128x32[:], rhs=stk[:], start=True, stop=True)
            stp_sb = ptiny.tile([1, 4], F32, tag="stpsb")
            nc.vector.tensor_copy(stp_sb[:], stp[0:1, :])
            mm = ptiny.tile([1, 2], F32, tag="mm")
            nc.vector.reduce_sum(mm[:], stp_sb[:].rearrange("p (a b) -> p b a", a=2),
                                 axis=mybir.AxisListType.X)
            mm2 = ptiny.tile([1, 2], F32, tag="mm2")
            nc.vector.tensor_scalar_mul(mm2[:], mm[:], 1.0 / DIM)  # [mu, E[x^2]]
            musq = ptiny.tile([1, 1], F32, tag="musq")
            nc.vector.tensor_tensor(musq[:], mm2[:, 0:1], mm2[:, 0:1], ALU.mult)
            var = ptiny.tile([1, 1], F32, tag="var")
            nc.vector.tensor_tensor(var[:], mm2[:, 1:2], musq[:], ALU.subtract)
            vare = ptiny.tile([1, 1], F32, tag="vare")
            nc.vector.tensor_scalar_add(vare[:], var[:], EPS)
            lnv = ptiny.tile([1, 1], F32, tag="lnv")
            nc.scalar.activation(lnv[:], vare[:], ACTF.Ln)
            s_t = ptiny.tile([1, 1], F32, tag="s")
            nc.scalar.activation(s_t[:], lnv[:], ACTF.Exp, scale=-0.5)
            sm = ptiny.tile([1, 2], F32, tag="sm")
            nc.vector.tensor_copy(sm[:, 0:1], s_t[:])
            nc.vector.tensor_copy(sm[:, 1:2], mm2[:, 0:1])
            bsm = up.tile([128, 2], F32, tag="u")
            nc.tensor.matmul(bsm[:], lhsT=ones1x128[:], rhs=sm[:], start=True, stop=True)
            # HAM warmup: bf16 burst on already-resident weights. In-order PE
            # FIFO places it right here -- after the stats matmuls, just
            # before the qkv projections -- lifting the clock gate to 2.4 GHz
            # for the whole qkv phase.
            warm = up.tile([128, 384], F32, tag="u")
            for w in range(14):
                nc.tensor.matmul(warm[:], lhsT=wqkvT[0][0:32, 0:128],
                                 rhs=wqkvT[0][0:32, :],
                                 start=True, stop=True, tile_position=(0, 0))
            a_c, d_c = [], []
            for kc in range(2):
                a = ptiny.tile([128, 1], F32, tag="ac", name=f"ac{kc}")
                nc.vector.tensor_tensor(a[:], gam_c[kc][:], bsm[:, 0:1], ALU.mult)
                a_c.append(a)
                t1 = ptiny.tile([128, 1], F32, tag="t1", name=f"t1{kc}")
                nc.vector.tensor_tensor(t1[:], a[:], bsm[:, 1:2], ALU.mult)
                d = ptiny.tile([128, 1], F32, tag="dc", name=f"dc{kc}")
                nc.vector.tensor_tensor(d[:], bet_c[kc][:], t1[:], ALU.subtract)
                d_c.append(d)

            # ---------- normalize + cast ----------
            xn3 = big.tile([128, 2, N], BF16, tag="xn")
            for half in range(2):
                for kc in range(2):
                    nc.vector.tensor_scalar(
                        xn3[:, kc, 2048 * half : 2048 * half + 2048],
                        xf[kc][:, 2048 * half : 2048 * half + 2048],
                        a_c[kc][:], d_c[kc][:], ALU.mult, ALU.add)

            # ---------- qkv projections ----------
            # K/Q packed (128, n): head h at partitions 32h..32h+31 -- row
            # tiling makes base 96 a legal operand position, so no separate
            # head-3 tiles. Q is pre-scaled via the folded identity.
            Kp = big.tile([128, N], BF16, tag="Kp")
            Qp = big.tile([128, NH], BF16, tag="Qp")
            vt3 = big.tile([128, 32, 33 * HEADS], BF16, tag="vt3")
            for nt in range(8):
                pk = simp.tile([128, 1024], F32, tag="sp")
                for kc in range(2):
                    nc.tensor.matmul(pk[:, 0:512], lhsT=wqkvT[kc][:, 128:256],
                                     rhs=xn3[:, kc, 512 * nt : 512 * nt + 512],
                                     start=(kc == 0), stop=(kc == 1))
                nc.vector.tensor_copy(Kp[:, 512 * nt : 512 * nt + 512], pk[:, 0:512])
            for nt in range(4):
                pq = simp.tile([128, 1024], F32, tag="sp")
                for kc in range(2):
                    nc.tensor.matmul(pq[:, 0:512], lhsT=wqkvT[kc][:, 0:128],
                                     rhs=xn3[:, kc, 512 * nt : 512 * nt + 512],
                                     start=(kc == 0), stop=(kc == 1))
                nc.vector.tensor_copy(Qp[:, 512 * nt : 512 * nt + 512], pq[:, 0:512])
            for jt in range(32):
                nc.gpsimd.memset(vt3[:, jt, :], 1.0)
                pv = simp.tile([128, 1024], F32, tag="sp")
                for kc in range(2):
                    nc.tensor.matmul(pv[:, 0:128],
                                     lhsT=xn3[:, kc, 128 * jt : 128 * jt + 128],
                                     rhs=wqkvT[kc][:, 256:384],
                                     start=(kc == 0), stop=(kc == 1))
                # one strided copy moves all 4 head blocks; dst keeps the
                # ones column at slot 32 of each 33-wide block, so head h's
                # channels land at cols 33h..33h+31 (ScalarE: frees DVE)
                dst = vt3[:, jt, :].rearrange("p (h c) -> p h c", h=HEADS)[:, :, 0:32]
                src = pv[:, 0:128].rearrange("p (h c) -> p h c", h=HEADS)
                nc.scalar.activation(dst, src, ACTF.Copy)

            # ---------- attention ----------
            # Per (t, jc): row-tiled sim quad -> exp (ACT/DVE split) ->
            # col-tiled attv pairs accumulating U in 2 PSUM banks.
            sched = _exp_schedule(4 * 32 * 2)

            if DEBUG:
                nc.sync.dma_start(dbg_kp[:, :], Kp[:])
                nc.sync.dma_start(dbg_qp[:, :], Qp[:])
                nc.sync.dma_start(dbg_vt[:, :],
                                  vt3[:].rearrange("p a b -> p (a b)"))

            # Per-t tail, software-pipelined INTO the next t's attention
            # stream so the PE never idles long enough for HAM to
            # re-throttle. Stages are separated so each stage's inputs are
            # several iterations old by the time its engine reaches it.
            def tail_gen(t, uev01, uev23):
                # stage 1: numerators + denominator pack (DVE, SBUF 4x)
                araw = puev.tile([128, 512], BF16, tag="araw", name=f"araw{t}")
                for h in range(HEADS):
                    src = (uev01 if h < 2 else uev23)
                    q = h % 2
                    nc.vector.tensor_copy(araw[32 * h : 32 * h + 32, :],
                                          src[64 * q : 64 * q + 32, :])
                dsm = puev.tile([1, 4 * 512], BF16, tag="dsm", name=f"dsm{t}")
                for h in range(HEADS):
                    src = (uev01 if h < 2 else uev23)
                    q = h % 2
                    nc.vector.tensor_copy(dsm[0:1, 512 * h : 512 * h + 512],
                                          src[64 * q + 32 : 64 * q + 33, :])
                yield
                # stage 2: spread denominators onto head bands + reciprocal
                # (pad allocations so the 3-slot sim rotation keeps phase)
                simp.tile([128, 1024], F32, tag="sp", name=f"pad2a_{t}")
                simp.tile([128, 1024], F32, tag="sp", name=f"pad2b_{t}")
                dp = simp.tile([128, 1024], F32, tag="sp")
                for h in range(HEADS):
                    nc.tensor.matmul(
                        dp[:, 0:512], lhsT=e_h[h][:],
                        rhs=dsm[0:1, 512 * h : 512 * h + 512],
                        start=(h == 0), stop=(h == 3),
                        tile_position=(0, 0), skip_group_check=True)
                rsb = pysb.tile([128, 512], F32, tag="rsb", name=f"rsb{t}")
                nc.vector.reciprocal_approx_fast(rsb[:], dp[:, 0:512])
                yield
                # stage 3: normalize
                at = puev.tile([128, 512], BF16, tag="araw", name=f"at{t}")
                if t == 3:
                    nc.vector.tensor_tensor(at[:], araw[:], rsb[:], ALU.mult)
                else:
                    nc.gpsimd.tensor_tensor(at[:], araw[:], rsb[:], ALU.mult)
                if DEBUG and t == 0:
                    nc.sync.dma_start(dbg_uev[:, 0:512], uev01[:])
                    nc.sync.dma_start(dbg_uev[:, 512:1024], uev23[:])
                    nc.sync.dma_start(dbg_rsb[:, :], rsb[:])
                    nc.sync.dma_start(dbg_at[:, :], at[:])
                yield
                # stage 4: output projection + bias + DMA
                simp.tile([128, 1024], F32, tag="sp", name=f"pad4_{t}")
                for kc in range(2):
                    py = simp.tile([128, 1024], F32, tag="sp")
                    nc.tensor.matmul(py[:, 0:512],
                                     lhsT=woT[:, 128 * kc : 128 * kc + 128],
                                     rhs=at[:], start=True, stop=True)
                    ysb = pysb.tile([128, 512], F32, tag="ysb")
                    if kc == 0:
                        nc.scalar.activation(ysb[:], py[:, 0:512], ACTF.Identity,
                                             bias=bo_c[kc][:])
                    else:
                        nc.vector.tensor_scalar(ysb[:], py[:, 0:512],
                                                bo_c[kc][:], None, ALU.add)
                    nc.sync.dma_start(
                        y_d[128 * kc : 128 * kc + 128, 512 * t : 512 * t + 512],
                        ysb[:])
                yield

            # Flat loop over all 128 (t, jc) iterations: the attv backlog
            # carries ACROSS t boundaries so the PE stream has no per-t
            # drain bubble (which was re-throttling the HAM clock gate).
            LAG = 4
            state = {"tail": None, "tick": 0}

            def attv(t, jc, exs, upair):
                for pair in range(2):
                    ex_bf = exs[pair].bitcast(BF16)
                    for q in range(2):
                        h = 2 * pair + q
                        nc.tensor.matmul(
                            upair[pair][64 * q : 64 * q + 33, :],
                            lhsT=vt3[:, jc, 33 * h : 33 * h + 33],
                            rhs=ex_bf[:, 512 * q : 512 * q + 512],
                            start=(jc == 0), stop=(jc == 31),
                            tile_position=(0, 64 * q),
                            skip_group_check=True)

            def pop_one(pending):
                t, jc, exs, upair = pending.pop(0)
                attv(t, jc, exs, upair)
                if jc == 31:
                    # t complete: evacuate U banks (frees the slots for the
                    # next t's accumulators) and start its pipelined tail
                    if state["tail"] is not None:
                        for _ in state["tail"]:
                            pass
                    uev01 = puev.tile([128, 512], BF16, tag="uev",
                                      name=f"uev01_{t}")
                    uev23 = puev.tile([128, 512], BF16, tag="uev",
                                      name=f"uev23_{t}")
                    nc.vector.tensor_copy(uev01[:], upair[0][:])
                    nc.vector.tensor_copy(uev23[:], upair[1][:])
                    state["tail"] = tail_gen(t, uev01, uev23)
                    state["tick"] = 0

            pending = []
            upair = None
            for g in range(128):
                t, jc = divmod(g, 32)
                if jc == 0:
                    u01 = up.tile([128, 512], F32, tag="u", name=f"u01_{t}")
                    u23 = up.tile([128, 512], F32, tag="u", name=f"u23_{t}")
                    upair = [u01, u23]
                if state["tail"] is not None:
                    state["tick"] += 1
                    if state["tick"] % 3 == 0:
                        next(state["tail"], None)
                stiles = [simp.tile([128, 1024], F32, tag="sp",
                                    name=f"sp{t}_{jc}_{k}") for k in range(2)]
                if jc >= 29 or (jc == 0 and g > 0):
                    for w in range(4):
                        nc.tensor.matmul(stiles[0][0:1, 0:64],
                                         lhsT=vt3[:, 0, 0:1],
                                         rhs=vt3[:, 0, 0:64],
                                         start=True, stop=True,
                                         tile_position=(0, 0),
                                         skip_group_check=True)
                for h in range(HEADS):
                    nc.tensor.matmul(
                        stiles[h // 2][:, 512 * (h % 2) : 512 * (h % 2) + 512],
                        lhsT=Kp[32 * h : 32 * h + 32, 128 * jc : 128 * jc + 128],
                        rhs=Qp[32 * h : 32 * h + 32, 512 * t : 512 * t + 512],
                        start=True, stop=True,
                        tile_position=(32 * h, 0))
                exs = []
                for pair in range(2):
                    ex = pexp.tile([128, 1024], I16, tag="ex")
                    if sched[g * 2 + pair]:
                        nc.scalar.activation(ex[:].bitcast(BF16),
                                             stiles[pair][:], ACTF.Exp,
                                             scale=SCALE)
                    else:
                        nc.vector.tensor_scalar(ex[:], stiles[pair][:],
                                                SCHR_A * SCALE, SCHR_B,
                                                ALU.mult, ALU.add)
                    exs.append(ex)
                if DEBUG and g == 0:
                    for pair in range(2):
                        sf = pysb.tile([128, 1024], F32, tag="ysb",
                                       name=f"dbgsim{pair}")
                        nc.vector.tensor_copy(sf[:], stiles[pair][:])
                        nc.sync.dma_start(
                            dbg_sim[:, 1024 * pair : 1024 * pair + 1024], sf[:])
                        nc.sync.dma_start(
                            dbg_ex[:, 1024 * pair : 1024 * pair + 1024],
                            exs[pair][:].bitcast(BF16))
                pending.append((t, jc, exs, upair))
                lag = 2 if g < 8 else LAG
                if len(pending) > lag and g % 2 == 1:
                    pop_one(pending)
                    pop_one(pending)
                    while len(pending) > lag + 1:
                        pop_one(pending)
                if g >= 118 and pending:
                    pop_one(pending)
            while pending:
                pop_one(pending)
            if state["tail"] is not None:
                for _ in state["tail"]:
                    pass
    nc.compile()
    return nc


_NC_CACHE = None


def get_nc():
    global _NC_CACHE
    if _NC_CACHE is None:
        _NC_CACHE = build_nc()
    return _NC_CACHE


def shard_inputs(x, gamma, beta, w_qkv, w_out, b_out):
    """Build the 8 per-core input maps (pure slicing / layout, no math)."""
    x = np.ascontiguousarray(np.asarray(x, dtype=np.float32))
    b, c, hh, ww = x.shape
    assert (b, c, hh, ww) == (4, DIM, 64, 64)
    xf = x.reshape(b, DIM, N)
    wq = np.ascontiguousarray(np.asarray(w_qkv, dtype=np.float32))
    wo = np.ascontiguousarray(np.asarray(w_out, dtype=np.float32))
    bo = np.asarray(b_out, dtype=np.float32).reshape(DIM, 1)
    gam = np.asarray(gamma, dtype=np.float32).reshape(DIM, 1)
    bet = np.asarray(beta, dtype=np.float32).reshape(DIM, 1)
    in_maps = []
    for core in range(N_CORES):
        bi, half = core // 2, core % 2
        xr = xf[bi] if half == 0 else np.roll(xf[bi], -NH, axis=1)
        in_maps.append({
            "xr": np.ascontiguousarray(xr),
            "wq": wq, "wo": wo, "bo": bo, "gam": gam, "bet": bet,
        })
    return in_maps


def gather_outputs(per_core_y):
    """per_core_y: list of 8 arrays (256, 2048) -> (4, 256, 64, 64) f32."""
    y = np.empty((4, DIM, N), dtype=np.float32)
    for core in range(N_CORES):
        bi, half = core // 2, core % 2
        y[bi][:, NH * half : NH * half + NH] = per_core_y[core]
    return y.reshape(4, DIM, 64, 64)


def kernel(x, gamma, beta, w_qkv, w_out, b_out):
    from concourse.bass_utils import run_bass_kernel_spmd

    nc = get_nc()
    in_maps = shard_inputs(x, gamma, beta, w_qkv, w_out, b_out)
    res = run_bass_kernel_spmd(nc, in_maps, core_ids=list(range(N_CORES)))
    return gather_outputs([res.results[c]["y"] for c in range(N_CORES)])


# revision 40
# speedup vs baseline: 1.0300x; 1.0300x over previous
"""Fused GroupNorm + multi-head self-attention + output projection for
nn_Attention_55619826483814 on 8 TRN2 NeuronCores.

Reference computation (shapes hardcoded):
  x: (4, 256, 64, 64) f32
  GroupNorm(1 group) over (C,H,W) per sample -> per-channel affine (gamma, beta)
  qkv = w_qkv @ xn  (384 = 3*4heads*32dim rows)
  per head: sim = (q*scale)^T k ; attn = softmax(sim, axis=j) ; out = attn @ v
  y = w_out @ out + b_out     -> (4, 256, 64, 64)

Sharding: 8 cores = 4 batches x 2 spatial halves. Core c handles batch c//2
and query positions [2048*(c%2), 2048*(c%2)+2048). Attention is permutation
invariant over key/value positions, so each core receives its batch image
with the spatial axis rolled so that ITS query half sits at columns 0..2047
(host-side np.roll = pure data movement). Keys/values/groupnorm stats use all
4096 positions. Each core computes its full (256, 2048) output slab; the host
concatenates. No collectives, no partition-id.

On-device dataflow per core (all matmul inputs bf16, fp32 accumulation):
  stats (DVE bn_stats/bn_aggr per 512-piece overlapping the x DMA + PE
  cross-partition sum; a bf16 HAM-warmup burst is emitted right after the
  stats matmuls so the in-order PE FIFO lifts the clock gate to 2.4 GHz
  just before the qkv projections)
  -> fold norm into per-channel a,d; cast x to bf16 xn = a*x + d
  -> Q,K packed channel-major (head h at partitions 32h..32h+31; the softmax
     SCALE is folded into exp's free scale operand) and V^T
     position-major with a ones column at slot 32 of every 33-wide head
     block (the attv col-tile then emits the softmax denominator as U row
     32 of its 33-partition output) via PE.
  -> attention, (t=i-tile, jc=j-chunk) loop:
     sim: 4 heads CONCURRENTLY via PE row tiling (tile_position=(32h,0),
       K=32 each) into two 2-bank PSUM tiles (h01 | h23), ~3x faster than
       serial K=32 matmuls.
     exp: split between ScalarE (exact Exp, bf16 out) and VectorE
       (Schraudolph approx: round(A*sim+B) via fp32->int16 convert, bitcast
       to bf16; |rel err| <= 3.3%, softmax ratio cancels most of it) with a
       static balance schedule -- both engines run flat out; this is the
       kernel's critical path (~33.5M exps/core, ~1 elem/lane/cycle each).
     attv: 2 heads CONCURRENTLY via PE col tiling (tile_position=(0,64q),
       M=33 each), accumulating U[den|ch, i] over jc in 2 PSUM banks.
  -> tail per t: evacuate U banks, spread denominators across partitions via
     K=1 indicator outer-products, one reciprocal, one multiply (GpSimd),
     y = w_out^T stationary @ attn^T + b_out -> DMA out. The whole attention
     runs as ONE flat 128-iteration loop: the attv backlog and the staged
     per-t tail carry across t boundaries so the PE stream never drains
     (a drain >3.4us re-throttles the HAM clock gate to 1.2 GHz, and at
     half clock the PE becomes the critical path). Tiny N=64 dummy matmuls
     pad the PE stream near boundaries; they write into the next sim tile
     BEFORE its start=True matmuls clear the bank, so they are free.

Measured: 262-272 us on 8 cores at full clock (baseline 496 us), rel err
7.6e-3 (gate 2e-2). Under the chip's P0 power-state downclock (sustained
load, 2.4->2.0 GHz) the same NEFF measures ~315 us; it recovers after a few
minutes idle. Engine balance in steady state: ScalarE ~90% (exact exp),
VectorE ~80% (Schraudolph exp + evac/normalize), PE ~80% (row-tiled sim +
col-tiled attv) -- co-critical at a ~1.26 us/iteration cadence.
"""

import sys

sys.path.insert(0, "/opt/trn_rl_repo")

import numpy as np

import concourse.bass as bass
import concourse.mybir as mybir
import concourse.tile as tile
from concourse import bacc
from concourse.masks import make_identity

DT = mybir.dt
F32 = DT.float32
BF16 = DT.bfloat16
I16 = DT.int16
ALU = mybir.AluOpType
ACTF = mybir.ActivationFunctionType

DIM = 256  # channels
N = 4096  # spatial positions
NH = 2048  # per-core query half
HEADS = 4
DH = 32  # head dim
HID = 128
SCALE = DH ** -0.5
EPS = 1e-5
NTOT = DIM * N

N_CORES = 8

# Schraudolph exp in bf16 bit domain: bits = round(A*x + B); DVE fp32->int16
# conversion rounds to nearest (hardware-verified). C=5.5 centers the
# sawtooth error at +-3.3%.
SCHR_A = float(2.0 ** 7 / np.log(2.0))
SCHR_B = float(127.0 * 128 - 5.5)

# Fraction of exp tiles handled by ScalarE (exact); rest go to VectorE
# (Schraudolph). Balances the two engines' total busy time.
ACT_FRAC = 0.57


def _exp_schedule(n_tiles):
    sched = []
    acc = 0.0
    for _ in range(n_tiles):
        acc += ACT_FRAC
        if acc >= 1.0:
            sched.append(True)
            acc -= 1.0
        else:
            sched.append(False)
    return sched


DEBUG = False


def build_nc():
    nc = bacc.Bacc("TRN2", target_bir_lowering=False)

    xr_d = nc.dram_tensor("xr", [DIM, N], F32, kind="ExternalInput")
    wq_d = nc.dram_tensor("wq", [3 * HID, DIM], F32, kind="ExternalInput")
    wo_d = nc.dram_tensor("wo", [DIM, HID], F32, kind="ExternalInput")
    bo_d = nc.dram_tensor("bo", [DIM, 1], F32, kind="ExternalInput")
    gam_d = nc.dram_tensor("gam", [DIM, 1], F32, kind="ExternalInput")
    bet_d = nc.dram_tensor("bet", [DIM, 1], F32, kind="ExternalInput")
    y_d = nc.dram_tensor("y", [DIM, NH], F32, kind="ExternalOutput")
    if DEBUG:
        dbg_kp = nc.dram_tensor("dbg_kp", [128, N], BF16, kind="ExternalOutput")
        dbg_qp = nc.dram_tensor("dbg_qp", [128, NH], BF16, kind="ExternalOutput")
        dbg_vt = nc.dram_tensor("dbg_vt", [128, 32 * 132], BF16, kind="ExternalOutput")
        dbg_sim = nc.dram_tensor("dbg_sim", [128, 2048], F32, kind="ExternalOutput")
        dbg_ex = nc.dram_tensor("dbg_ex", [128, 2048], BF16, kind="ExternalOutput")
        dbg_uev = nc.dram_tensor("dbg_uev", [128, 1024], BF16, kind="ExternalOutput")
        dbg_rsb = nc.dram_tensor("dbg_rsb", [128, 512], F32, kind="ExternalOutput")
        dbg_at = nc.dram_tensor("dbg_at", [128, 512], BF16, kind="ExternalOutput")

    with tile.TileContext(nc) as tc:
        with (
            tc.tile_pool(name="small", bufs=1) as small,
            tc.tile_pool(name="big", bufs=1) as big,
            tc.tile_pool(name="pxf", bufs=2) as pxf,
            tc.tile_pool(name="pjunk", bufs=1) as pjunk,
            tc.tile_pool(name="pwst", bufs=3) as pwst,
            tc.tile_pool(name="pwost", bufs=2) as pwost,
            tc.tile_pool(name="ptiny", bufs=2) as ptiny,
            tc.tile_pool(name="puev", bufs=2) as puev,
            tc.tile_pool(name="pexp", bufs=12) as pexp,
            tc.tile_pool(name="pysb", bufs=4) as pysb,
            tc.tile_pool(name="simp", bufs=3, space="PSUM") as simp,
            tc.tile_pool(name="up", bufs=2, space="PSUM") as up,
        ):
            # ---------- constants ----------
            identity = small.tile([128, 128], F32, tag="ident")
            make_identity(nc, identity[:])
            ones128x32 = small.tile([128, 32], F32, tag="o12832")
            nc.gpsimd.memset(ones128x32[:], 1.0)
            ones1x128 = small.tile([1, 128], F32, tag="o1128")
            nc.gpsimd.memset(ones1x128[:], 1.0)
            # indicator rows e_h[0, m] = (m // 32 == h): a K=1 matmul with
            # lhsT=e_h replicates a (1, n) rhs onto partitions 32h..32h+31
            e_h = []
            for h in range(HEADS):
                e = small.tile([1, 128], BF16, tag=f"e{h}", name=f"e{h}")
                nc.gpsimd.memset(e[:], 0.0)
                nc.gpsimd.memset(e[:, 32 * h : 32 * h + 32], 1.0)
                e_h.append(e)

            gam_c, bet_c, bo_c = [], [], []
            for kc in range(2):
                g = small.tile([128, 1], F32, tag=f"gam{kc}", name=f"gam{kc}")
                nc.sync.dma_start(g[:], gam_d[128 * kc : 128 * kc + 128, :])
                gam_c.append(g)
                bt = small.tile([128, 1], F32, tag=f"bet{kc}", name=f"bet{kc}")
                nc.sync.dma_start(bt[:], bet_d[128 * kc : 128 * kc + 128, :])
                bet_c.append(bt)
                bb = small.tile([128, 1], F32, tag=f"bo{kc}", name=f"bo{kc}")
                nc.sync.dma_start(bb[:], bo_d[128 * kc : 128 * kc + 128, :])
                bo_c.append(bb)

            # ---------- load x (4 pieces per chunk so stats overlap DMA) ----------
            xf = []
            for kc in range(2):
                t = pxf.tile([128, N], F32, tag="xf", name=f"xf{kc}")
                for p in range(4):
                    nc.sync.dma_start(
                        t[:, 1024 * p : 1024 * p + 1024],
                        xr_d[128 * kc : 128 * kc + 128, 1024 * p : 1024 * p + 1024])
                xf.append(t)

            # ---------- weight transposes ----------
            wqkvT = [big.tile([128, 384], BF16, tag=f"wqkvT{c}", name=f"wqkvT{c}") for c in range(2)]
            for r in range(3):
                wst = pwst.tile([128, DIM], F32, tag="wst")
                nc.sync.dma_start(wst[:], wq_d[128 * r : 128 * r + 128, :])
                for c in range(2):
                    tp = up.tile([128, 128], F32, tag="u")
                    nc.tensor.transpose(tp[:], wst[:, 128 * c : 128 * c + 128], identity[:])
                    nc.vector.tensor_copy(wqkvT[c][:, 128 * r : 128 * r + 128], tp[:])
            woT = big.tile([128, DIM], BF16, tag="woT")
            for r in range(2):
                wst = pwost.tile([128, HID], F32, tag="wost")
                nc.sync.dma_start(wst[:], wo_d[128 * r : 128 * r + 128, :])
                tp = up.tile([128, 128], F32, tag="u")
                nc.tensor.transpose(tp[:], wst[:], identity[:])
                nc.vector.tensor_copy(woT[:, 128 * r : 128 * r + 128], tp[:])

            # ---------- groupnorm stats (bn_stats: one DVE pass/piece) ----------
            bno = ptiny.tile([128, 2, 8, 6], F32, tag="bno")
            for kc in range(2):
                for p in range(8):
                    piece = xf[kc][:, 512 * p : 512 * p + 512]
                    nc.vector.bn_stats(bno[:, kc, p], piece)
            # per-channel [mean, var] over each 4096-wide half
            stk = ptiny.tile([128, 4], F32, tag="stk")  # [m0, s0, m1, s1]
            for kc in range(2):
                mv = ptiny.tile([128, 2], F32, tag="mv", name=f"mv{kc}")
                nc.vector.bn_aggr(mv[:], bno[:, kc])
                nc.vector.tensor_copy(stk[:, 2 * kc : 2 * kc + 1], mv[:, 0:1])
                # s = E[x^2] = var + mean^2
                msq = ptiny.tile([128, 1], F32, tag="msq", name=f"msq{kc}")
                nc.vector.tensor_tensor(msq[:], mv[:, 0:1], mv[:, 0:1], ALU.mult)
                nc.vector.tensor_tensor(stk[:, 2 * kc + 1 : 2 * kc + 2],
                                        mv[:, 1:2], msq[:], ALU.add)
            stp = up.tile([32, 4], F32, tag="u")
            nc.tensor.matmul(stp[:], lhsT=ones128x32[:], rhs=stk[:], start=True, stop=True)
            stp_sb = ptiny.tile([1, 4], F32, tag="stpsb")
            nc.vector.tensor_copy(stp_sb[:], stp[0:1, :])
            mm = ptiny.tile([1, 2], F32, tag="mm")
            nc.vector.reduce_sum(mm[:], stp_sb[:].rearrange("p (a b) -> p b a", a=2),
                                 axis=mybir.AxisListType.X)
            mm2 = ptiny.tile([1, 2], F32, tag="mm2")
            nc.vector.tensor_scalar_mul(mm2[:], mm[:], 1.0 / DIM)  # [mu, E[x^2]]
            musq = ptiny.tile([1, 1], F32, tag="musq")
            nc.vector.tensor_tensor(musq[:], mm2[:, 0:1], mm2[:, 0:1], ALU.mult)
            var = ptiny.tile([1, 1], F32, tag="var")
            nc.vector.tensor_tensor(var[:], mm2[:, 1:2], musq[:], ALU.subtract)
            vare = ptiny.tile([1, 1], F32, tag="vare")
            nc.vector.tensor_scalar_add(vare[:], var[:], EPS)
            lnv = ptiny.tile([1, 1], F32, tag="lnv")
            nc.scalar.activation(lnv[:], vare[:], ACTF.Ln)
            s_t = ptiny.tile([1, 1], F32, tag="s")
            nc.scalar.activation(s_t[:], lnv[:], ACTF.Exp, scale=-0.5)
            sm = ptiny.tile([1, 2], F32, tag="sm")
            nc.vector.tensor_copy(sm[:, 0:1], s_t[:])
            nc.vector.tensor_copy(sm[:, 1:2], mm2[:, 0:1])
            bsm = up.tile([128, 2], F32, tag="u")
            nc.tensor.matmul(bsm[:], lhsT=ones1x128[:], rhs=sm[:], start=True, stop=True)
            # HAM warmup: bf16 burst on already-resident weights. In-order PE
            # FIFO places it right here -- after the stats matmuls, just
            # before the qkv projections -- lifting the clock gate to 2.4 GHz
            # for the whole qkv phase.
            warm = up.tile([128, 384], F32, tag="u")
            for w in range(14):
                nc.tensor.matmul(warm[:], lhsT=wqkvT[0][0:32, 0:128],
                                 rhs=wqkvT[0][0:32, :],
                                 start=True, stop=True, tile_position=(0, 0))
            a_c, d_c = [], []
            for kc in range(2):
                a = ptiny.tile([128, 1], F32, tag="ac", name=f"ac{kc}")
                nc.vector.tensor_tensor(a[:], gam_c[kc][:], bsm[:, 0:1], ALU.mult)
                a_c.append(a)
                t1 = ptiny.tile([128, 1], F32, tag="t1", name=f"t1{kc}")
                nc.vector.tensor_tensor(t1[:], a[:], bsm[:, 1:2], ALU.mult)
                d = ptiny.tile([128, 1], F32, tag="dc", name=f"dc{kc}")
                nc.vector.tensor_tensor(d[:], bet_c[kc][:], t1[:], ALU.subtract)
                d_c.append(d)

            # ---------- normalize + cast ----------
            xn3 = big.tile([128, 2, N], BF16, tag="xn")
            for half in range(2):
                for kc in range(2):
                    nc.vector.tensor_scalar(
                        xn3[:, kc, 2048 * half : 2048 * half + 2048],
                        xf[kc][:, 2048 * half : 2048 * half + 2048],
                        a_c[kc][:], d_c[kc][:], ALU.mult, ALU.add)

            # ---------- qkv projections ----------
            # K/Q packed (128, n): head h at partitions 32h..32h+31 -- row
            # tiling makes base 96 a legal operand position, so no separate
            # head-3 tiles. Q is pre-scaled via the folded identity.
            Kp = big.tile([128, N], BF16, tag="Kp")
            Qp = big.tile([128, NH], BF16, tag="Qp")
            vt3 = big.tile([128, 32, 33 * HEADS], BF16, tag="vt3")
            for nt in range(8):
                pk = simp.tile([128, 1024], F32, tag="sp")
                for kc in range(2):
                    nc.tensor.matmul(pk[:, 0:512], lhsT=wqkvT[kc][:, 128:256],
                                     rhs=xn3[:, kc, 512 * nt : 512 * nt + 512],
                                     start=(kc == 0), stop=(kc == 1))
                nc.vector.tensor_copy(Kp[:, 512 * nt : 512 * nt + 512], pk[:, 0:512])
            for nt in range(4):
                pq = simp.tile([128, 1024], F32, tag="sp")
                for kc in range(2):
                    nc.tensor.matmul(pq[:, 0:512], lhsT=wqkvT[kc][:, 0:128],
                                     rhs=xn3[:, kc, 512 * nt : 512 * nt + 512],
                                     start=(kc == 0), stop=(kc == 1))
                nc.vector.tensor_copy(Qp[:, 512 * nt : 512 * nt + 512], pq[:, 0:512])
            for jt in range(32):
                nc.gpsimd.memset(vt3[:, jt, :], 1.0)
                pv = simp.tile([128, 1024], F32, tag="sp")
                for kc in range(2):
                    nc.tensor.matmul(pv[:, 0:128],
                                     lhsT=xn3[:, kc, 128 * jt : 128 * jt + 128],
                                     rhs=wqkvT[kc][:, 256:384],
                                     start=(kc == 0), stop=(kc == 1))
                # one strided copy moves all 4 head blocks; dst keeps the
                # ones column at slot 32 of each 33-wide block, so head h's
                # channels land at cols 33h..33h+31 (ScalarE: frees DVE)
                dst = vt3[:, jt, :].rearrange("p (h c) -> p h c", h=HEADS)[:, :, 0:32]
                src = pv[:, 0:128].rearrange("p (h c) -> p h c", h=HEADS)
                nc.scalar.activation(dst, src, ACTF.Copy)

            # ---------- attention ----------
            # Per (t, jc): row-tiled sim quad -> exp (ACT/DVE split) ->
            # col-tiled attv pairs accumulating U in 2 PSUM banks.
            sched = _exp_schedule(4 * 32 * 2)

            if DEBUG:
                nc.sync.dma_start(dbg_kp[:, :], Kp[:])
                nc.sync.dma_start(dbg_qp[:, :], Qp[:])
                nc.sync.dma_start(dbg_vt[:, :],
                                  vt3[:].rearrange("p a b -> p (a b)"))

            # Per-t tail, software-pipelined INTO the next t's attention
            # stream so the PE never idles long enough for HAM to
            # re-throttle. Stages are separated so each stage's inputs are
            # several iterations old by the time its engine reaches it.
            def tail_gen(t, uev01, uev23):
                # stage 1: numerators + denominator pack (DVE, SBUF 4x)
                araw = puev.tile([128, 512], BF16, tag="araw", name=f"araw{t}")
                for h in range(HEADS):
                    src = (uev01 if h < 2 else uev23)
                    q = h % 2
                    nc.vector.tensor_copy(araw[32 * h : 32 * h + 32, :],
                                          src[64 * q : 64 * q + 32, :])
                dsm = puev.tile([1, 4 * 512], BF16, tag="dsm", name=f"dsm{t}")
                for h in range(HEADS):
                    src = (uev01 if h < 2 else uev23)
                    q = h % 2
                    nc.vector.tensor_copy(dsm[0:1, 512 * h : 512 * h + 512],
                                          src[64 * q + 32 : 64 * q + 33, :])
                yield
                # stage 2: spread denominators onto head bands + reciprocal
                # (pad allocations so the 3-slot sim rotation keeps phase)
                simp.tile([128, 1024], F32, tag="sp", name=f"pad2a_{t}")
                simp.tile([128, 1024], F32, tag="sp", name=f"pad2b_{t}")
                dp = simp.tile([128, 1024], F32, tag="sp")
                for h in range(HEADS):
                    nc.tensor.matmul(
                        dp[:, 0:512], lhsT=e_h[h][:],
                        rhs=dsm[0:1, 512 * h : 512 * h + 512],
                        start=(h == 0), stop=(h == 3),
                        tile_position=(0, 0), skip_group_check=True)
                rsb = pysb.tile([128, 512], F32, tag="rsb", name=f"rsb{t}")
                nc.vector.reciprocal_approx_fast(rsb[:], dp[:, 0:512])
                yield
                # stage 3: normalize
                at = puev.tile([128, 512], BF16, tag="araw", name=f"at{t}")
                if t == 3:
                    nc.vector.tensor_tensor(at[:], araw[:], rsb[:], ALU.mult)
                else:
                    nc.gpsimd.tensor_tensor(at[:], araw[:], rsb[:], ALU.mult)
                if DEBUG and t == 0:
                    nc.sync.dma_start(dbg_uev[:, 0:512], uev01[:])
                    nc.sync.dma_start(dbg_uev[:, 512:1024], uev23[:])
                    nc.sync.dma_start(dbg_rsb[:, :], rsb[:])
                    nc.sync.dma_start(dbg_at[:, :], at[:])
                yield
                # stage 4: output projection + bias + DMA
                simp.tile([128, 1024], F32, tag="sp", name=f"pad4_{t}")
                for kc in range(2):
                    py = simp.tile([128, 1024], F32, tag="sp")
                    nc.tensor.matmul(py[:, 0:512],
                                     lhsT=woT[:, 128 * kc : 128 * kc + 128],
                                     rhs=at[:], start=True, stop=True)
                    ysb = pysb.tile([128, 512], F32, tag="ysb")
                    if kc == 0:
                        nc.scalar.activation(ysb[:], py[:, 0:512], ACTF.Identity,
                                             bias=bo_c[kc][:])
                    else:
                        nc.vector.tensor_scalar(ysb[:], py[:, 0:512],
                                                bo_c[kc][:], None, ALU.add)
                    nc.sync.dma_start(
                        y_d[128 * kc : 128 * kc + 128, 512 * t : 512 * t + 512],
                        ysb[:])
                yield

            # Flat loop over all 128 (t, jc) iterations: the attv backlog
            # carries ACROSS t boundaries so the PE stream has no per-t
            # drain bubble (which was re-throttling the HAM clock gate).
            LAG = 4
            state = {"tail": None, "tick": 0}

            def attv(t, jc, exs, upair):
                for pair in range(2):
                    ex_bf = exs[pair].bitcast(BF16)
                    for q in range(2):
                        h = 2 * pair + q
                        nc.tensor.matmul(
                            upair[pair][64 * q : 64 * q + 33, :],
                            lhsT=vt3[:, jc, 33 * h : 33 * h + 33],
                            rhs=ex_bf[:, 512 * q : 512 * q + 512],
                            start=(jc == 0), stop=(jc == 31),
                            tile_position=(0, 64 * q),
                            skip_group_check=True)

            def pop_one(pending):
                t, jc, exs, upair = pending.pop(0)
                attv(t, jc, exs, upair)
                if jc == 31:
                    # t complete: evacuate U banks (frees the slots for the
                    # next t's accumulators) and start its pipelined tail
                    if state["tail"] is not None:
                        for _ in state["tail"]:
                            pass
                    uev01 = puev.tile([128, 512], BF16, tag="uev",
                                      name=f"uev01_{t}")
                    uev23 = puev.tile([128, 512], BF16, tag="uev",
                                      name=f"uev23_{t}")
                    nc.vector.tensor_copy(uev01[:], upair[0][:])
                    nc.vector.tensor_copy(uev23[:], upair[1][:])
                    state["tail"] = tail_gen(t, uev01, uev23)
                    state["tick"] = 0

            pending = []
            upair = None
            for g in range(128):
                t, jc = divmod(g, 32)
                if jc == 0:
                    u01 = up.tile([128, 512], F32, tag="u", name=f"u01_{t}")
                    u23 = up.tile([128, 512], F32, tag="u", name=f"u23_{t}")
                    upair = [u01, u23]
                if state["tail"] is not None:
                    state["tick"] += 1
                    if state["tick"] % 3 == 0:
                        next(state["tail"], None)
                stiles = [simp.tile([128, 1024], F32, tag="sp",
                                    name=f"sp{t}_{jc}_{k}") for k in range(2)]
                if jc >= 29 or (jc == 0 and g > 0):
                    for w in range(4):
                        nc.tensor.matmul(stiles[0][0:1, 0:64],
                                         lhsT=vt3[:, 0, 0:1],
                                         rhs=vt3[:, 0, 0:64],
                                         start=True, stop=True,
                                         tile_position=(0, 0),
                                         skip_group_check=True)
                for h in range(HEADS):
                    nc.tensor.matmul(
                        stiles[h // 2][:, 512 * (h % 2) : 512 * (h % 2) + 512],
                        lhsT=Kp[32 * h : 32 * h + 32, 128 * jc : 128 * jc + 128],
                        rhs=Qp[32 * h : 32 * h + 32, 512 * t : 512 * t + 512],
                        start=True, stop=True,
                        tile_position=(32 * h, 0))
                exs = []
                for pair in range(2):
                    ex = pexp.tile([128, 1024], I16, tag="ex")
                    if sched[g * 2 + pair]:
                        nc.scalar.activation(ex[:].bitcast(BF16),
                                             stiles[pair][:], ACTF.Exp,
                                             scale=SCALE)
                    else:
                        nc.vector.tensor_scalar(ex[:], stiles[pair][:],
                                                SCHR_A * SCALE, SCHR_B,
                                                ALU.mult, ALU.add)
                    exs.append(ex)
                if DEBUG and g == 0:
                    for pair in range(2):
                        sf = pysb.tile([128, 1024], F32, tag="ysb",
                                       name=f"dbgsim{pair}")
                        nc.vector.tensor_copy(sf[:], stiles[pair][:])
                        nc.sync.dma_start(
                            dbg_sim[:, 1024 * pair : 1024 * pair + 1024], sf[:])
                        nc.sync.dma_start(
                            dbg_ex[:, 1024 * pair : 1024 * pair + 1024],
                            exs[pair][:].bitcast(BF16))
                pending.append((t, jc, exs, upair))
                lag = 2 if g < 8 else LAG
                if len(pending) > lag and g % 2 == 1:
                    pop_one(pending)
                    pop_one(pending)
                    while len(pending) > lag + 1:
                        pop_one(pending)
                if g >= 118 and pending:
                    pop_one(pending)
            while pending:
                pop_one(pending)
            if state["tail"] is not None:
                for _ in state["tail"]:
                    pass
    nc.compile()
    return nc


_NC_CACHE = None


def get_nc():
    global _NC_CACHE
    if _NC_CACHE is None:
        _NC_CACHE = build_nc()
    return _NC_CACHE


def shard_inputs(x, gamma, beta, w_qkv, w_out, b_out):
    """Build the 8 per-core input maps (pure slicing / layout, no math)."""
    x = np.ascontiguousarray(np.asarray(x, dtype=np.float32))
    b, c, hh, ww = x.shape
    assert (b, c, hh, ww) == (4, DIM, 64, 64)
    xf = x.reshape(b, DIM, N)
    wq = np.ascontiguousarray(np.asarray(w_qkv, dtype=np.float32))
    wo = np.ascontiguousarray(np.asarray(w_out, dtype=np.float32))
    bo = np.asarray(b_out, dtype=np.float32).reshape(DIM, 1)
    gam = np.asarray(gamma, dtype=np.float32).reshape(DIM, 1)
    bet = np.asarray(beta, dtype=np.float32).reshape(DIM, 1)
    in_maps = []
    for core in range(N_CORES):
        bi, half = core // 2, core % 2
        xr = xf[bi] if half == 0 else np.roll(xf[bi], -NH, axis=1)
        in_maps.append({
            "xr": np.ascontiguousarray(xr),
            "wq": wq, "wo": wo, "bo": bo, "gam": gam, "bet": bet,
        })
    return in_maps


def gather_outputs(per_core_y):
    """per_core_y: list of 8 arrays (256, 2048) -> (4, 256, 64, 64) f32."""
    y = np.empty((4, DIM, N), dtype=np.float32)
    for core in range(N_CORES):
        bi, half = core // 2, core % 2
        y[bi][:, NH * half : NH * half + NH] = per_core_y[core]
    return y.reshape(4, DIM, 64, 64)


def kernel(x, gamma, beta, w_qkv, w_out, b_out):
    from concourse.bass_utils import run_bass_kernel_spmd

    nc = get_nc()
    in_maps = shard_inputs(x, gamma, beta, w_qkv, w_out, b_out)
    res = run_bass_kernel_spmd(nc, in_maps, core_ids=list(range(N_CORES)))
    return gather_outputs([res.results[c]["y"] for c in range(N_CORES)])
